# revision 15
# baseline (speedup 1.0000x reference)
"""GQA attention kernel for 8 Trainium2 NeuronCores.

Sharding: tensor-parallel over heads. Core i handles query heads (2i, 2i+1)
and KV head i//2. Out-proj is row-parallel: each core emits a partial
[S, DIM] output; the host sums the 8 partials and adds the output bias.

v3: all heavy streams bf16 (v2), plus a HAM-aware schedule. The PE clock
gate (HAM) throttles to 4/8 pulses when the PE idles >~3.4us, so the
kernel emits one dense PE stream: projection chunk sc, then attention for
query chunk sc-1, then its out-proj slice, round-robin. RoPE/bias (ACT,
DVE) for chunk sc overlap the attention matmuls of chunk sc-1. Input DMAs
load contract-dim pairs ([128, 2, 512] = 2KB per partition line) to halve
issue count; output rows stage in SBUF and fly as one DMA per 128 rows.

On-chip layouts keep head_dim (128) on partitions and sequence on the free
axis, so QK^T needs no transposes, softmax statistics are PE ones-matmuls,
and the attention weights feed the AV matmul directly from the exp output.
"""

import numpy as np

DIM = 2048
Q_HEADS = 16
KV_HEADS = 4
HEAD_DIM = 128
S = 2048
MAX_LEN = 2048
ROPE_THETA = 10000.0
ROPE_FACTOR = 8.0
N_CORES = 8
HEADS_PER_CORE = Q_HEADS // N_CORES  # 2
SCALE = 1.0 / np.sqrt(HEAD_DIM)
NEG = -1.0e30

_F32R_CACHE = {}


def _bf16(x):
    import ml_dtypes

    return np.ascontiguousarray(np.asarray(x, np.float32).astype(ml_dtypes.bfloat16))


def _rope_cos_sin_T():
    d = HEAD_DIM
    seq_eff = max(S, MAX_LEN)
    base_adj = (ROPE_FACTOR * seq_eff / MAX_LEN - (ROPE_FACTOR - 1.0)) ** (d / (d - 2))
    adjusted_base = ROPE_THETA * base_adj
    inv_freq = 1.0 / adjusted_base ** (np.arange(0, d, 2, dtype=np.float32) / d)
    pos = np.arange(S, dtype=np.float32)
    freqs = pos[:, None] * inv_freq[None, :]
    emb = np.concatenate([freqs, freqs], axis=-1)  # [S, d]
    return (
        np.ascontiguousarray(np.cos(emb).T.astype(np.float32)),  # [d, S]
        np.ascontiguousarray(np.sin(emb).T.astype(np.float32)),
    )


def _masks():
    # additive masks for the 4 diagonal 128x512 blocks: block r covers keys
    # [128r, 128r+128) against queries [0, 512) within a 512-query chunk.
    k = np.arange(128)[:, None]
    q = np.arange(512)[None, :]
    m = np.zeros((128, 4, 512), np.float32)
    for r in range(4):
        m[:, r, :] = np.where(128 * r + k > q, NEG, 0.0).astype(np.float32)
    return np.ascontiguousarray(m.reshape(128, 4 * 512))


def _build_program():
    import concourse.bass as bass
    import concourse.tile as tile
    from concourse import mybir
    import bass_rust
    from concourse.vector_clock import ScopedClock
    from concourse.masks import make_identity

    # --- workaround: walrus CTRL instructions accept a single sync wait;
    # split the TileContext end-drain waits across one SP nop each.
    def _patched_drain_and_barrier(self, tick_clock, wait_clock):
        nop0 = self.nc.sync.nop(nofuse=True)
        wait_clock.add_sem_waits(nop0.ins, ScopedClock({None: tick_clock.global_clock}))
        si = nop0.ins.sync_info
        ws = list(si.on_wait) if si is not None else []
        if len(ws) > 1:
            nop0.ins.sync_info = bass_rust.SyncInfo(
                on_wait=ws[:1], on_update=list(si.on_update))
            for i in range(1, len(ws)):
                nop = self.nc.sync.nop(nofuse=True)
                nop.ins.sync_info = bass_rust.SyncInfo(on_wait=ws[i:i + 1], on_update=[])
        self.nc.sync.drain()
        self.nc.all_engine_barrier()
        popped = self.nc._tile_sem_poison_stack.pop()
        assert popped is self._sem_poison
        self.nc.clear_and_free_semaphores(list(self.sems.allocated().values()))
        self.nc.all_engine_barrier()

    tile.TileContext._drain_and_barrier = _patched_drain_and_barrier

    def _split_multi_waits(nc):
        # this walrus build accepts a single sync-wait slot on several
        # instruction encodings; peel extra waits onto same-engine NoOps.
        cnt = 0
        for f in nc.m.functions:
            for bb in f.blocks:
                new_l = []
                for inst in bb.instructions:
                    si = inst.sync_info
                    ws = list(si.on_wait) if si is not None else []
                    if len(ws) > 1:
                        for w in ws[:-1]:
                            nop = mybir.InstNoOp(
                                name=f"{inst.name}_wsplit{cnt}", engine=inst.engine,
                                bass_nofuse=True,
                                sync_info=mybir.SyncInfo(on_wait=[w], on_update=[]))
                            nc.register_instruction(nop, overwrite=True)
                            new_l.append(nop)
                            cnt += 1
                        inst.sync_info = mybir.SyncInfo(
                            on_wait=[ws[-1]], on_update=list(si.on_update))
                    new_l.append(inst)
                bb.instructions = new_l

    f32 = mybir.dt.float32
    bf16 = mybir.dt.bfloat16
    AF = mybir.ActivationFunctionType
    OP = mybir.AluOpType

    nc = bass.Bass()
    qT_in = nc.dram_tensor("queryT", [DIM, S], bf16, kind="ExternalInput")
    kT_in = nc.dram_tensor("keyT", [DIM, S], bf16, kind="ExternalInput")
    vT_in = nc.dram_tensor("valueT", [DIM, S], bf16, kind="ExternalInput")
    wq_in = nc.dram_tensor("wqT", [DIM, 256], bf16, kind="ExternalInput")
    wk_in = nc.dram_tensor("wkT", [DIM, 128], bf16, kind="ExternalInput")
    wv_in = nc.dram_tensor("wvT", [DIM, 128], bf16, kind="ExternalInput")
    wo_in = nc.dram_tensor("woT", [256, DIM], bf16, kind="ExternalInput")
    bq_in = nc.dram_tensor("bq_col", [128, 2], f32, kind="ExternalInput")
    bk_in = nc.dram_tensor("bk_col", [128, 1], f32, kind="ExternalInput")
    bv_in = nc.dram_tensor("bv_col", [128, 1], f32, kind="ExternalInput")
    cos_in = nc.dram_tensor("cosT", [128, S], f32, kind="ExternalInput")
    sin_in = nc.dram_tensor("sinT", [128, S], f32, kind="ExternalInput")
    mask_in = nc.dram_tensor("masks", [128, 4 * 512], f32, kind="ExternalInput")
    out_dram = nc.dram_tensor("partial", [S, DIM], bf16, kind="ExternalOutput")

    # contract-dim pair view: row = cp*256 + two*128 + ci
    qT_r = qT_in.rearrange("(cp two ci) s -> ci cp two s", ci=128, two=2)
    kT_r = kT_in.rearrange("(cp two ci) s -> ci cp two s", ci=128, two=2)
    vT_r = vT_in.rearrange("(cp two ci) s -> ci cp two s", ci=128, two=2)

    with tile.TileContext(nc) as tc:
        with (
            tc.tile_pool(name="const", bufs=1) as cpool,
            tc.tile_pool(name="stream", bufs=4) as spool,
            tc.tile_pool(name="work", bufs=2) as wpool,
            tc.tile_pool(name="acts", bufs=1) as apool,
            tc.tile_pool(name="attn", bufs=2) as atpool,
            tc.tile_pool(name="ps1", bufs=1, space="PSUM") as ps1,
            tc.tile_pool(name="ps2", bufs=2, space="PSUM") as ps2,
        ):
            # ---- constants / weights. Chunked + spread across the sync,
            # scalar, and vector DGE queues so the first projection matmuls
            # aren't gated on a 9MB single-queue preload: proj weights lead
            # on sync/scalar (ahead of the input streams on those queues),
            # bulk constants (cos/sin/masks/wo, needed 25-50us in) go to the
            # otherwise-idle vector queue.
            wq_r = wq_in.rearrange("(co ci) d -> ci co d", ci=128)
            wk_r = wk_in.rearrange("(co ci) d -> ci co d", ci=128)
            wv_r = wv_in.rearrange("(co ci) d -> ci co d", ci=128)
            wq_sb = cpool.tile([128, 16, 256], bf16)
            wk_sb = cpool.tile([128, 16, 128], bf16)
            wv_sb = cpool.tile([128, 16, 128], bf16)
            nc.sync.dma_start(wq_sb[:, 0:4], wq_r[:, 0:4])
            nc.scalar.dma_start(wk_sb[:, 0:8], wk_r[:, 0:8])
            nc.sync.dma_start(wv_sb[:, 0:8], wv_r[:, 0:8])
            nc.scalar.dma_start(wq_sb[:, 4:8], wq_r[:, 4:8])
            nc.sync.dma_start(wq_sb[:, 8:12], wq_r[:, 8:12])
            nc.scalar.dma_start(wq_sb[:, 12:16], wq_r[:, 12:16])
            nc.scalar.dma_start(wk_sb[:, 8:16], wk_r[:, 8:16])
            nc.sync.dma_start(wv_sb[:, 8:16], wv_r[:, 8:16])
            cos_sb = cpool.tile([128, S], f32)
            nc.gpsimd.dma_start(cos_sb[:], cos_in[:])
            sin_sb = cpool.tile([128, S], f32)
            nc.gpsimd.dma_start(sin_sb[:], sin_in[:])
            bq_sb = cpool.tile([128, 2], f32)
            nc.gpsimd.dma_start(bq_sb[:], bq_in[:])
            bk_sb = cpool.tile([128, 1], f32)
            nc.gpsimd.dma_start(bk_sb[:], bk_in[:])
            bv_sb = cpool.tile([128, 1], f32)
            nc.gpsimd.dma_start(bv_sb[:], bv_in[:])
            mask_sb = cpool.tile([128, 4, 512], f32)
            nc.gpsimd.dma_start(mask_sb[:], mask_in.rearrange("p (r q) -> p r q", r=4))
            wo_sb = cpool.tile([128, 2, DIM], bf16)
            wo_r = wo_in.rearrange("(h d) e -> d h e", d=128)
            nc.gpsimd.dma_start(wo_sb[:, 0], wo_r[:, 0])
            nc.gpsimd.dma_start(wo_sb[:, 1], wo_r[:, 1])
            ones_f = cpool.tile([128, 128], f32)
            nc.vector.memset(ones_f[:], 1.0)
            ones_mat = cpool.tile([128, 128], bf16)
            nc.vector.tensor_copy(out=ones_mat[:], in_=ones_f[:])
            ident = cpool.tile([128, 128], f32)
            make_identity(nc, ident[:])

            # ---- persistent per-chunk activations (bf16 matmul operands)
            q_rot = [[apool.tile([128, 512], bf16, tag=f"qrot{h}_{c}", name=f"qrot{h}_{c}")
                      for c in range(4)] for h in range(2)]
            k_rot = [apool.tile([128, 512], bf16, tag=f"krot{c}", name=f"krot{c}")
                     for c in range(4)]
            v_sb = [apool.tile([128, 512], bf16, tag=f"vsb{c}", name=f"vsb{c}")
                    for c in range(4)]
            ctxT = [[apool.tile([128, 512], bf16, tag=f"ctx{h}_{c}", name=f"ctx{h}_{c}")
                     for c in range(4)] for h in range(2)]

            def rope(dst, raw, sc):
                # dst = raw*cos + swap(raw)*sinMod; sinMod has the -1 on the
                # low half baked in host-side (rotate_half sign).
                # math in f32, single rounding into the bf16 dst.
                ssl = slice(sc * 512, sc * 512 + 512)
                swp = wpool.tile([128, 512], f32, tag="ropeswp")
                nc.vector.tensor_copy(out=swp[0:64, :], in_=raw[64:128, :])
                nc.vector.tensor_copy(out=swp[64:128, :], in_=raw[0:64, :])
                tmp = wpool.tile([128, 512], f32, tag="ropetmp")
                nc.vector.tensor_tensor(tmp[:], swp[:], sin_sb[:, ssl], OP.mult)
                acc = wpool.tile([128, 512], f32, tag="ropeacc")
                nc.vector.tensor_tensor(acc[:], raw[:], cos_sb[:, ssl], OP.mult)
                nc.vector.tensor_tensor(acc[:], acc[:], tmp[:], OP.add)
                nc.vector.tensor_copy(out=dst[:], in_=acc[:])

            def proj_mm(sc):
                # projection matmuls for sequence chunk sc; returns live psums
                ssl = slice(sc * 512, sc * 512 + 512)
                pq0 = ps1.tile([128, 512], f32, tag="A")
                pq1 = ps1.tile([128, 512], f32, tag="B")
                pk = ps1.tile([128, 512], f32, tag="C")
                pv = ps1.tile([128, 512], f32, tag="D")
                for cp in range(8):
                    qt = spool.tile([128, 2, 512], bf16, tag="qs")
                    nc.sync.dma_start(qt[:], qT_r[:, cp, :, ssl])
                    kt_ = spool.tile([128, 2, 512], bf16, tag="ks")
                    nc.scalar.dma_start(kt_[:], kT_r[:, cp, :, ssl])
                    vt = spool.tile([128, 2, 512], bf16, tag="vs")
                    nc.sync.dma_start(vt[:], vT_r[:, cp, :, ssl])
                    for t in range(2):
                        cc = 2 * cp + t
                        st, sp = cc == 0, cc == 15
                        nc.tensor.matmul(pq0[:], wq_sb[:, cc, 0:128],
                                         qt[:, t], start=st, stop=sp)
                        nc.tensor.matmul(pq1[:], wq_sb[:, cc, 128:256],
                                         qt[:, t], start=st, stop=sp)
                        nc.tensor.matmul(pk[:], wk_sb[:, cc],
                                         kt_[:, t], start=st, stop=sp)
                        nc.tensor.matmul(pv[:], wv_sb[:, cc],
                                         vt[:, t], start=st, stop=sp)
                return pq0, pq1, pk, pv

            def proj_epilogue(sc, pq0, pq1, pk, pv):
                # bias + RoPE (q, k) on ACT/DVE; bias + transpose (v)
                q0_raw = wpool.tile([128, 512], f32, tag="raw")
                nc.scalar.activation(q0_raw[:], pq0[:], AF.Identity, bias=bq_sb[:, 0:1])
                rope(q_rot[0][sc], q0_raw, sc)
                q1_raw = wpool.tile([128, 512], f32, tag="raw")
                nc.scalar.activation(q1_raw[:], pq1[:], AF.Identity, bias=bq_sb[:, 1:2])
                rope(q_rot[1][sc], q1_raw, sc)
                k_raw = wpool.tile([128, 512], f32, tag="raw")
                nc.scalar.activation(k_raw[:], pk[:], AF.Identity, bias=bk_sb[:])
                rope(k_rot[sc], k_raw, sc)
                v_raw = wpool.tile([128, 512], f32, tag="raw")
                nc.scalar.activation(v_raw[:], pv[:], AF.Identity, bias=bv_sb[:])
                for j in range(4):
                    ptr = ps1.tile([128, 128], f32, tag="A")
                    nc.tensor.transpose(ptr[:], v_raw[:, j * 128:(j + 1) * 128], ident[:])
                    nc.vector.tensor_copy(
                        out=v_sb[sc][:, j * 128:j * 128 + 128], in_=ptr[:])

            def attn(qc):
                # attention for query chunk qc, both heads
                n_kt = 4 * (qc + 1)
                for h in range(2):
                    attnT = atpool.tile([128, 16, 512], bf16, tag="attnT")
                    for kt in range(n_kt):
                        pst = ps2.tile([128, 512], f32, tag="sT")
                        nc.tensor.matmul(
                            pst[:], k_rot[kt // 4][:, (kt % 4) * 128:(kt % 4) * 128 + 128],
                            q_rot[h][qc][:], start=True, stop=True)
                        r = kt - 4 * qc
                        if r >= 0:
                            nc.vector.tensor_tensor(pst[:], pst[:], mask_sb[:, r], OP.add)
                        nc.scalar.activation(attnT[:, kt], pst[:], AF.Exp, scale=float(SCALE))
                    psum = ps1.tile([128, 512], f32, tag="C")
                    pctx = ps1.tile([128, 512], f32, tag="B")
                    for kt in range(n_kt):
                        nc.tensor.matmul(psum[:], ones_mat[:],
                                         attnT[:, kt],
                                         start=kt == 0, stop=kt == n_kt - 1)
                        nc.tensor.matmul(pctx[:], v_sb[kt // 4][:, (kt % 4) * 128:(kt % 4) * 128 + 128],
                                         attnT[:, kt],
                                         start=kt == 0, stop=kt == n_kt - 1)
                    bc_sb = wpool.tile([128, 512], f32, tag="bc")
                    nc.vector.reciprocal(out=bc_sb[:], in_=psum[:])
                    nc.vector.tensor_tensor(ctxT[h][qc][:], pctx[:], bc_sb[:], OP.mult)

            def outproj(qc):
                # out-proj rows for the 4 seq tiles of query chunk qc
                for st in range(4 * qc, 4 * qc + 4):
                    tsl = slice((st % 4) * 128, (st % 4) * 128 + 128)
                    ot = wpool.tile([128, 2048], bf16, tag="ot")
                    for ec in range(4):
                        esl = slice(ec * 512, ec * 512 + 512)
                        po = ps2.tile([128, 512], f32, tag="po")
                        nc.tensor.matmul(po[:], ctxT[0][qc][:, tsl],
                                         wo_sb[:, 0, esl], start=True, stop=False)
                        nc.tensor.matmul(po[:], ctxT[1][qc][:, tsl],
                                         wo_sb[:, 1, esl], start=False, stop=True)
                        if ec % 2 == 0:
                            nc.vector.tensor_copy(out=ot[:, esl], in_=po[:])
                        else:
                            nc.scalar.activation(ot[:, esl], po[:], AF.Copy)
                    nc.sync.dma_start(
                        out_dram[st * 128:st * 128 + 128, :], ot[:])

            # ---- HAM-aware interleave: keep the PE stream dense.
            # proj(0), epi(0), proj(1), epi(1)+attn(0)+out(0), proj(2), ...
            ps = proj_mm(0)
            proj_epilogue(0, *ps)
            for sc in range(1, 4):
                ps = proj_mm(sc)
                proj_epilogue(sc, *ps)
                attn(sc - 1)
                outproj(sc - 1)
            attn(3)
            outproj(3)
    _split_multi_waits(nc)
    return nc


def kernel(query, key, value, Wq, bq, Wk, bk, Wv, bv, Wo, bo):
    from concourse.bass_utils import run_bass_kernel_spmd

    query = np.asarray(query, np.float32)
    key = np.asarray(key, np.float32)
    value = np.asarray(value, np.float32)
    B = query.shape[0]
    qT = _bf16(query.reshape(S, DIM).T)
    kT = _bf16(key.reshape(S, DIM).T)
    vT = _bf16(value.reshape(S, DIM).T)
    cosT, sinT = _rope_cos_sin_T()
    sinT = sinT.copy()
    sinT[0:64, :] *= -1.0  # rotate_half: low half gets -x2*sin
    sinT = np.ascontiguousarray(sinT)
    masks = _masks()

    if "nc" not in _F32R_CACHE:
        _F32R_CACHE["nc"] = _build_program()
    nc = _F32R_CACHE["nc"]

    in_maps = []
    for i in range(N_CORES):
        g = i // 2
        Wq_s = _bf16(np.asarray(Wq, np.float32)[256 * i:256 * (i + 1), :].T)
        Wk_s = _bf16(np.asarray(Wk, np.float32)[128 * g:128 * (g + 1), :].T)
        Wv_s = _bf16(np.asarray(Wv, np.float32)[128 * g:128 * (g + 1), :].T)
        Wo_s = _bf16(np.asarray(Wo, np.float32)[:, 256 * i:256 * (i + 1)].T)
        bq_c = np.ascontiguousarray(np.asarray(bq, np.float32)[256 * i:256 * (i + 1)].reshape(2, 128).T)
        bk_c = np.asarray(bk, np.float32)[128 * g:128 * (g + 1)].reshape(128, 1)
        bv_c = np.asarray(bv, np.float32)[128 * g:128 * (g + 1)].reshape(128, 1)
        in_maps.append({
            "queryT": qT, "keyT": kT, "valueT": vT,
            "wqT": Wq_s, "wkT": Wk_s, "wvT": Wv_s, "woT": Wo_s,
            "bq_col": bq_c, "bk_col": np.ascontiguousarray(bk_c),
            "bv_col": np.ascontiguousarray(bv_c),
            "cosT": cosT, "sinT": sinT, "masks": masks,
        })

    _F32R_CACHE["in_maps"] = in_maps
    globals()["_LAST_IN_MAPS"] = in_maps
    res = run_bass_kernel_spmd(nc, in_maps, list(range(N_CORES)))
    out = res.results[0]["partial"].astype(np.float32)
    for i in range(1, N_CORES):
        out = out + res.results[i]["partial"].astype(np.float32)
    out = out + np.asarray(bo, np.float32)[None, :]
    return out.reshape(B, S, DIM).astype(np.float32)
